# revision 12
# baseline (speedup 1.0000x reference)
"""V6: single-collective Chebyshev attention, restructured data paths.

Rank-1 scores S[i,j] = q_i*k_j collapse softmax-attention to two scalar
functions per batch:
    g(s) = sum_j exp(s*k_j)            Z_i  = g(q_i)
    f(t) = sum_i (v_i/Z_i) exp(q_i*t)  sa_j = f(k_j)
Both are least-squares degree-7 polynomial fits through 32 Chebyshev nodes
(host-side Vandermonde pinv, consistent with the bf16-rounded node
positions), evaluated with one scalar_tensor_tensor per Horner step and an
even/odd split to halve the dependency depth.

V6 structural changes over V5 (50072ns):
- host-side tiled x/W DRAM layouts -> contiguous per-partition loads:
  x in ONE HWDGE dma, W in 4 chunks alternating Pool/DVE SWDGE queues
  (SWDGE costs ~1us fixed per dma_start on the issuing engine).
- small consts packed into 3 HWDGE dmas instead of 8 Pool SWDGE dmas.
- bias matmul FIRST (start=True), q|k|v merged into one [32,768] matmul
  per contraction chunk.
- post-collective: ONE [32,768] cc_sb load; the arg broadcasts become 8
  per-source-core matmuls reading cc_sb directly; q/k/v point layouts
  hop off cc_sb SBUF-side.  krow/qrow/vrow/qp4-from-DRAM all deleted.
- fscr (f node accumulation) split DVE/Pool with two accum_outs.
"""
import numpy as np
from contextlib import ExitStack

import concourse.bass as bass
from concourse import bacc, mybir
import concourse.tile as tile
from concourse.bass_utils import run_bass_kernel_spmd

F = mybir.ActivationFunctionType
DT = mybir.dt
OP = mybir.AluOpType

SEQ = 2048
B = 32
NCORES = 8
SL = SEQ // NCORES      # 256 features per core
BL = B // NCORES        # 4 batches per core post-collective
KCH = SEQ // 128        # 16 contraction chunks
NCH = 32                # chebyshev nodes
R = 8                   # polynomial terms (degree 7)
TQ = 3.5                # q-domain half-width
TK = 3.0                # k-domain half-width
N_WARM1 = 188           # PE keep-hot dummies spanning the collective
N_WARM2 = 38            # PE keep-hot dummies spanning the w roundtrip
N_WARM0 = 12            # PE pre-warm before the projection stream
FSPL = 1280             # fscr split: DVE gets [0:FSPL], Pool the rest

_CACHE = {}


def _consts():
    import ml_dtypes
    bf16 = ml_dtypes.bfloat16
    m = np.arange(NCH)
    u = np.cos(np.pi * (m + 0.5) / NCH)
    # node masks live in bf16 (matmul dtype parity with the bf16 rows), so
    # use the bf16-ROUNDED node positions and build least-squares
    # values->monomial maps consistent with those exact nodes.
    tqn = np.asarray(TQ * u, dtype=bf16).astype(np.float64)   # g nodes
    tkn = np.asarray(TK * u, dtype=bf16).astype(np.float64)   # f nodes

    def v2mono(nodes_scaled):
        V = np.vander(nodes_scaled, R, increasing=True)       # [NCH, R]
        return np.linalg.pinv(V)                              # [R, NCH]

    Kq = v2mono(tqn / TQ)   # g: coeffs in u = q/TQ from values at tqn/TQ
    Kk = v2mono(tkn / TK)   # f: coeffs in u = k/TK from values at tkn/TK
    krhs = np.zeros((128, 2 * R), np.float32)  # [(i,m), j] = K[j, m]; g|f
    mask01 = np.zeros((128, 128), np.float32)  # [(i,m),(i',pp)] = (i==i')
    tqmask = np.zeros((BL, 128), bf16)         # [i',(i,m)] = (i==i')*tqn_m
    tkmask = np.zeros((BL, 128), bf16)
    bmask = np.zeros((BL, 128), bf16)          # [i',(i,m)] = (i==i')
    for i in range(BL):
        krhs[32 * i:32 * i + 32, 0:R] = Kq.T.astype(np.float32)
        krhs[32 * i:32 * i + 32, R:2 * R] = Kk.T.astype(np.float32)
        mask01[32 * i:32 * i + 32, 32 * i:32 * i + 32] = 1.0
        tqmask[i, 32 * i:32 * i + 32] = tqn.astype(bf16)
        tkmask[i, 32 * i:32 * i + 32] = tkn.astype(bf16)
        bmask[i, 32 * i:32 * i + 32] = 1.0
    return krhs, mask01, tqmask, tkmask, bmask


def _build():
    nc = bacc.Bacc("TRN2", target_bir_lowering=False, debug=False,
                   num_devices=NCORES)
    # tiled layouts: per-partition contiguous
    xT_d = nc.dram_tensor("xT", [128, KCH * B], DT.bfloat16,
                          kind="ExternalInput")
    w_d = nc.dram_tensor("w", [128, KCH * 3 * SL], DT.float8e4,
                         kind="ExternalInput")
    # packed consts: bf16 row (ones|bias), f32 block (krhs|mask01|xp4),
    # bf16 mask block (tqm|tkm|bm)
    cb_d = nc.dram_tensor("cb", [1, B + 3 * SL], DT.bfloat16,
                          kind="ExternalInput")
    cf_d = nc.dram_tensor("cf", [128, 2 * R + 128 + 64], DT.float32,
                          kind="ExternalInput")
    cm_d = nc.dram_tensor("cm", [BL, 3 * 128], DT.bfloat16,
                          kind="ExternalInput")
    out_d = nc.dram_tensor("out", [BL, SEQ], DT.float32, kind="ExternalOutput")

    cc_in = nc.dram_tensor("cc_in", [B, 3 * SL], DT.bfloat16)
    cc_out = nc.dram_tensor("cc_out", [B, 3 * SL], DT.bfloat16)

    H = SEQ // 2
    Q = SEQ // 4

    with tile.TileContext(nc) as tc, ExitStack() as ctx:
        pool = ctx.enter_context(tc.tile_pool(name="main", bufs=1))

        # ---- phase 1 loads ------------------------------------------------
        # x: one HWDGE dma (contiguous per partition)
        xt = pool.tile([128, KCH * B], DT.bfloat16)
        nc.sync.dma_start(xt[:], xT_d.ap())
        # consts: three HWDGE dmas
        cb_t = pool.tile([1, B + 3 * SL], DT.bfloat16)
        nc.sync.dma_start(cb_t[:], cb_d.ap())
        cf_t = pool.tile([128, 2 * R + 128 + 64], DT.float32)
        nc.sync.dma_start(cf_t[:], cf_d.ap())
        cm_t = pool.tile([BL, 3 * 128], DT.bfloat16)
        nc.sync.dma_start(cm_t[:], cm_d.ap())
        ones_t = cb_t[:, 0:B]
        bias_t = cb_t[:, B:B + 3 * SL]
        krhs_t = cf_t[:, 0:2 * R]
        mask_t = cf_t[:, 2 * R:2 * R + 128]
        xp4 = cf_t[:, 2 * R + 128:2 * R + 192]
        tqm_t = cm_t[:, 0:128]
        tkm_t = cm_t[:, 128:256]
        bm_t = cm_t[:, 256:384]

        # W: 4 chunks of 4 contraction-chunks each, alternating Pool/DVE
        # SWDGE queues (pipelines the DMA_ENGINES stream)
        wall = pool.tile([128, KCH * 3 * SL], DT.float8e4)
        WCH = 4 * 3 * SL  # columns per W chunk
        for c, eng in ((0, nc.gpsimd), (1, nc.scalar),
                       (2, nc.gpsimd), (3, nc.scalar)):
            eng.dma_start(wall[:, c * WCH:(c + 1) * WCH],
                          w_d.ap()[:, c * WCH:(c + 1) * WCH])

        warm = pool.tile([1, 1], DT.float32)
        nc.scalar.activation(warm[:], ones_t[0:1, 0:1], F.Exp)

        # PE pre-warm on a memset scratch so the projection matmuls price at
        # full clock (p-state is locked at visit; cold PE runs 3.7x slower)
        scratch = pool.tile([128, SL], DT.bfloat16)
        nc.vector.memset(scratch[:], 1.0)
        warm_ctx = ExitStack()
        pwx = warm_ctx.enter_context(tc.tile_pool(name="pswarm", bufs=1,
                                                  space="PSUM"))
        scr = pwx.tile([B, SL], DT.float32)
        for d in range(N_WARM0):
            nc.tensor.matmul(scr[:], scratch[:, 0:B], scratch[:],
                             start=(d == 0), stop=False)

        # ---- phase 1 compute: bias first, then merged q|k|v per chunk ----
        cvt = pool.tile([B, 3 * SL], DT.bfloat16)
        with tc.tile_pool(name="psp", bufs=1, space="PSUM") as pp:
            ps_qk = pp.tile([B, 2 * SL], DT.float32)
            ps_v = pp.tile([B, SL], DT.float32)
            nc.tensor.matmul(ps_qk[:], ones_t[:], bias_t[:, 0:2 * SL],
                             start=True, stop=False)
            nc.tensor.matmul(ps_v[:], ones_t[:], bias_t[:, 2 * SL:3 * SL],
                             start=True, stop=False)
            for kc in range(KCH):
                xk = xt[:, kc * B:(kc + 1) * B]
                nc.tensor.matmul(ps_qk[:], xk,
                                 wall[:, kc * 768:kc * 768 + 512],
                                 start=False, stop=(kc == KCH - 1))
                nc.tensor.matmul(ps_v[:], xk,
                                 wall[:, kc * 768 + 512:(kc + 1) * 768],
                                 start=False, stop=(kc == KCH - 1))
            # parallel converts (undo the x64 fp8 weight scaling)
            nc.scalar.activation(cvt[:, 2 * SL:3 * SL], ps_v[:], F.Copy,
                                 scale=1.0 / 64.0)
            nc.vector.tensor_scalar(cvt[:, 0:2 * SL], ps_qk[:], 1.0 / 64.0,
                                    None, op0=OP.mult)
        nc.sync.dma_start(cc_in.ap(), cvt[:])
        nc.gpsimd.collective_compute(
            "AllToAll", OP.bypass, replica_groups=[list(range(NCORES))],
            ins=[cc_in.ap()], outs=[cc_out.ap()])

        # keep PE hot through the collective window so post-collective
        # matmuls are costed at full clock (p-state is locked at visit time)
        for d in range(N_WARM1):
            nc.tensor.matmul(scr[:], scratch[:, 0:B], scratch[:],
                             start=False, stop=(d == N_WARM1 - 1))
        warm_ctx.close()

        # ---- phase 2: one batch-major cc load; broadcasts read it direct --
        # cc2[i, (d, o)] = cc_out[(d, i), o]: every per-source-core slice
        # starts at partition 0 (PE rhs base-partition restriction)
        cc2 = pool.tile([BL, NCORES * 3 * SL], DT.bfloat16)
        nc.sync.dma_start(
            cc2[:].rearrange("i (d o) -> i d o", d=NCORES),
            cc_out.ap().rearrange("(d i) o -> i d o", i=BL))
        # point layouts per-batch direct from DRAM (3D in-APs; a single 4D
        # AP pairing is rejected by the 3-dim DMA balancing limit).
        # p4[(i, pp), f] = row[i, pp*64+f] with pp = d*4 + o4.
        cco = cc_out.ap()
        qp4 = pool.tile([128, 64], DT.bfloat16)
        for i in range(BL):
            nc.scalar.dma_start(
                qp4[32 * i:32 * i + 32, :],
                cco[:, 0:SL].rearrange("(d i) (o4 f) -> i d o4 f",
                                       i=BL, f=64)[i])
        kp4 = pool.tile([128, 64], DT.bfloat16)
        for i in range(BL):
            nc.sync.dma_start(
                kp4[32 * i:32 * i + 32, :],
                cco[:, SL:2 * SL].rearrange("(d i) (o4 f) -> i d o4 f",
                                            i=BL, f=64)[i])
        v4 = pool.tile([128, 64], DT.bfloat16)
        for i in range(BL):
            nc.gpsimd.dma_start(
                v4[32 * i:32 * i + 32, :],
                cco[:, 2 * SL:3 * SL].rearrange("(d i) (o4 f) -> i d o4 f",
                                                i=BL, f=64)[i])

        # u and s = u^2 tiles
        uq = pool.tile([128, 64], DT.float32)
        nc.vector.tensor_scalar(uq[:], qp4[:], 1.0 / TQ, None, op0=OP.mult)
        sq = pool.tile([128, 64], DT.float32)
        nc.vector.tensor_mul(sq[:], uq[:], uq[:])
        uk = pool.tile([128, 64], DT.float32)
        nc.vector.tensor_scalar(uk[:], kp4[:], 1.0 / TK, None, op0=OP.mult)
        sk = pool.tile([128, 64], DT.float32)
        nc.vector.tensor_mul(sk[:], uk[:], uk[:])

        def horner(co, s, u, extra, name):
            """P(u) = sum_j co_j u^j, even/odd split; adds `extra` if given."""
            te = pool.tile([128, 64], DT.float32, name=f"te_{name}")
            to = pool.tile([128, 64], DT.float32, name=f"to_{name}")
            nc.vector.tensor_scalar(te[:], s[:], co[:, 6:7], None,
                                    op0=OP.mult)
            nc.vector.tensor_scalar(to[:], s[:], co[:, 7:8], None,
                                    op0=OP.mult)
            for j in (4, 2):
                nc.vector.scalar_tensor_tensor(
                    te[:], te[:], co[:, j:j + 1], s[:], OP.add, OP.mult)
                nc.vector.scalar_tensor_tensor(
                    to[:], to[:], co[:, j + 1:j + 2], s[:], OP.add, OP.mult)
            nc.vector.scalar_tensor_tensor(
                to[:], to[:], co[:, 1:2], u[:], OP.add, OP.mult)
            res = pool.tile([128, 64], DT.float32, name=f"res_{name}")
            if extra is None:
                nc.vector.tensor_scalar(te[:], te[:], co[:, 0:1], None,
                                        op0=OP.add)
            else:
                nc.vector.scalar_tensor_tensor(
                    te[:], te[:], co[:, 0:1], extra[:], OP.add, OP.add)
            nc.vector.tensor_add(res[:], te[:], to[:])
            return res

        gscr = pool.tile([128, SEQ], DT.bfloat16)
        gv = pool.tile([128, 1], DT.float32)
        p4 = pool.tile([128, SEQ], DT.bfloat16)
        fscr = pool.tile([128, SEQ], DT.bfloat16)
        fvq = [pool.tile([128, 1], DT.float32, name=f"fvq{q}")
               for q in range(4)]
        fva = pool.tile([128, 1], DT.float32)
        fvb = pool.tile([128, 1], DT.float32)
        fv = pool.tile([128, 1], DT.float32)

        with tc.tile_pool(name="psbig", bufs=1, space="PSUM") as pb:
            karg = pb.tile([128, SEQ], DT.float32)
            qarg = pb.tile([128, SEQ], DT.float32)
            # arg[(i,m), j] = t_m * row_i[j]: per-source-core outer products
            # reading cc_sb directly (k slice = cols SL..2SL, q = 0..SL)
            for d in range(NCORES):
                nc.tensor.matmul(karg[:, d * SL:(d + 1) * SL], tqm_t,
                                 cc2[:, d * 768 + SL:d * 768 + 2 * SL],
                                 start=True, stop=True)
            for d in range(NCORES):
                nc.tensor.matmul(qarg[:, d * SL:(d + 1) * SL], tkm_t,
                                 cc2[:, d * 768:d * 768 + SL],
                                 start=True, stop=True)
            # g node values: gv[(i,m)] = sum_j exp(karg)
            nc.scalar.activation(gscr[:], karg[:], F.Exp, accum_out=gv[:])
            # f exp table
            nc.scalar.activation(p4[:], qarg[:], F.Exp)

            # ---- g: Z at q-points, w = v/Z --------------------------------
            gvm = pool.tile([128, 128], DT.float32)
            nc.vector.tensor_scalar(gvm[:], mask_t, gv[:, 0:1], None,
                                    op0=OP.mult)
            cog = pool.tile([128, R], DT.float32)
            # mono matmul lands in spare karg columns (gexp already read them)
            nc.tensor.matmul(karg[:, SEQ - R:SEQ], gvm[:], krhs_t[:, 0:R],
                             start=True, stop=True)
            nc.vector.tensor_copy(cog[:], karg[:, SEQ - R:SEQ])

            # keep PE hot until the w broadcast (deterministic LOW pricing)
            for d in range(N_WARM2):
                nc.tensor.matmul(karg[:, 0:SL], tqm_t, cc2[:, SL:2 * SL],
                                 start=True, stop=True)

            zt = horner(cog, sq, uq, None, "g")
            rz = pool.tile([128, 64], DT.float32)
            nc.vector.reciprocal(rz[:], zt[:])
            wt = pool.tile([128, 64], DT.bfloat16)
            nc.vector.tensor_mul(wt[:], v4[:], rz[:])

            # w: point -> row layout (one SBUF->SBUF hop) -> PE broadcast
            # into the karg banks (gexp is done with them)
            wflat = pool.tile([BL, SEQ], DT.bfloat16)
            nc.sync.dma_start(wflat[:], wt[:])
            for q in range(4):
                nc.tensor.matmul(karg[:, q * Q:(q + 1) * Q], bm_t,
                                 wflat[:, q * Q:(q + 1) * Q],
                                 start=True, stop=True)
            # fv[(i,m)] = sum_j p4 * w4: four DVE chunks pipelined behind the
            # w-broadcast quarters (Pool cannot read PSUM), then combine
            for q in range(4):
                nc.vector.scalar_tensor_tensor(
                    fscr[:, q * Q:(q + 1) * Q], p4[:, q * Q:(q + 1) * Q],
                    1.0, karg[:, q * Q:(q + 1) * Q],
                    OP.mult, OP.mult, accum_out=fvq[q][:])
            nc.vector.tensor_add(fva[:], fvq[0][:], fvq[1][:])
            nc.vector.tensor_add(fvb[:], fvq[2][:], fvq[3][:])
            nc.vector.tensor_add(fv[:], fva[:], fvb[:])

            # f mono coeffs via the same spare-column trick (qarg this time)
            fvm = pool.tile([128, 128], DT.float32)
            nc.vector.tensor_scalar(fvm[:], mask_t, fv[:, 0:1], None,
                                    op0=OP.mult)
            cof = pool.tile([128, R], DT.float32)
            nc.tensor.matmul(qarg[:, SEQ - R:SEQ], fvm[:],
                             krhs_t[:, R:2 * R], start=True, stop=True)
            nc.vector.tensor_copy(cof[:], qarg[:, SEQ - R:SEQ])

        # ---- f: sa at k-points + residual ---------------------------------
        so = horner(cof, sk, uk, xp4, "f")
        nc.sync.dma_start(
            out_d.ap().rearrange("i (pp f) -> (i pp) f", f=64), so[:])
    nc.compile()
    return nc


def _prep_inputs(x, Wq, bq, Wk, bk, Wv, bv):
    import ml_dtypes
    bf16 = ml_dtypes.bfloat16
    fp8 = ml_dtypes.float8_e4m3
    x = np.ascontiguousarray(x, dtype=np.float32)
    # tiled xT: [p, kc*B+m] = x[m, kc*128+p]
    xT = np.ascontiguousarray(
        x.T.astype(bf16).reshape(KCH, 128, B).transpose(1, 0, 2)
        .reshape(128, KCH * B))
    krhs, mask01, tqmask, tkmask, bmask = _consts()
    ones = np.ones((1, B), dtype=bf16)
    cm = np.ascontiguousarray(
        np.concatenate([tqmask, tkmask, bmask], axis=1))
    in_maps = []
    for c in range(NCORES):
        sl = slice(SL * c, SL * (c + 1))
        w_all = np.concatenate([Wq[sl].T, Wk[sl].T, Wv[sl].T], axis=1)
        w_tiled = np.ascontiguousarray(
            (w_all * 64.0).astype(fp8).reshape(KCH, 128, 3 * SL)
            .transpose(1, 0, 2).reshape(128, KCH * 3 * SL))
        bias = np.concatenate([bq[sl], bk[sl], bv[sl]])[None, :]
        cb = np.ascontiguousarray(np.concatenate(
            [ones, (bias * 64.0).astype(bf16)], axis=1, dtype=bf16))
        xloc = x[BL * c:BL * (c + 1)]
        xp4 = np.ascontiguousarray(xloc.reshape(128, 64))
        cf = np.ascontiguousarray(
            np.concatenate([krhs, mask01, xp4], axis=1, dtype=np.float32))
        in_maps.append({
            "xT": xT,
            "w": w_tiled,
            "cb": cb,
            "cf": cf,
            "cm": cm,
        })
    return in_maps


def run_on_device(x, Wq, bq, Wk, bk, Wv, bv, **spmd_kwargs):
    if "nc" not in _CACHE:
        _CACHE["nc"] = _build()
    nc = _CACHE["nc"]
    in_maps = _prep_inputs(x, Wq, bq, Wk, bk, Wv, bv)
    res = run_bass_kernel_spmd(nc, in_maps, core_ids=list(range(NCORES)),
                               **spmd_kwargs)
    out = np.concatenate([res.results[c]["out"] for c in range(NCORES)], axis=0)
    return np.ascontiguousarray(out, dtype=np.float32), res


def kernel(x, Wq, bq, Wk, bk, Wv, bv):
    out, _ = run_on_device(x, Wq, bq, Wk, bk, Wv, bv)
    return out


# revision 16
# speedup vs baseline: 1.0186x; 1.0186x over previous
"""V6: single-collective Chebyshev attention, restructured data paths.

Rank-1 scores S[i,j] = q_i*k_j collapse softmax-attention to two scalar
functions per batch:
    g(s) = sum_j exp(s*k_j)            Z_i  = g(q_i)
    f(t) = sum_i (v_i/Z_i) exp(q_i*t)  sa_j = f(k_j)
Both are least-squares degree-7 polynomial fits through 32 Chebyshev nodes
(host-side Vandermonde pinv, consistent with the bf16-rounded node
positions), evaluated with one scalar_tensor_tensor per Horner step and an
even/odd split to halve the dependency depth.

V6 structural changes over V5 (50072ns):
- host-side tiled x/W DRAM layouts -> contiguous per-partition loads:
  x in ONE HWDGE dma, W in 4 chunks alternating Pool/DVE SWDGE queues
  (SWDGE costs ~1us fixed per dma_start on the issuing engine).
- small consts packed into 3 HWDGE dmas instead of 8 Pool SWDGE dmas.
- bias matmul FIRST (start=True), q|k|v merged into one [32,768] matmul
  per contraction chunk.
- post-collective: ONE [32,768] cc_sb load; the arg broadcasts become 8
  per-source-core matmuls reading cc_sb directly; q/k/v point layouts
  hop off cc_sb SBUF-side.  krow/qrow/vrow/qp4-from-DRAM all deleted.
- fscr (f node accumulation) split DVE/Pool with two accum_outs.
"""
import numpy as np
from contextlib import ExitStack

import concourse.bass as bass
from concourse import bacc, mybir
import concourse.tile as tile
from concourse.bass_utils import run_bass_kernel_spmd

F = mybir.ActivationFunctionType
DT = mybir.dt
OP = mybir.AluOpType

SEQ = 2048
B = 32
NCORES = 8
SL = SEQ // NCORES      # 256 features per core
BL = B // NCORES        # 4 batches per core post-collective
KCH = SEQ // 128        # 16 contraction chunks
NCH = 32                # chebyshev nodes
R = 8                   # polynomial terms (degree 7)
TQ = 3.5                # q-domain half-width
TK = 3.0                # k-domain half-width
N_WARM1 = 164           # PE keep-hot dummies spanning the collective
N_WARM2 = 20            # PE keep-hot dummies spanning the w roundtrip
N_WARM0 = 16            # PE pre-warm before the projection stream
FSPL = 1280             # fscr split: DVE gets [0:FSPL], Pool the rest

_CACHE = {}


def _consts():
    import ml_dtypes
    bf16 = ml_dtypes.bfloat16
    m = np.arange(NCH)
    u = np.cos(np.pi * (m + 0.5) / NCH)
    # node masks live in bf16 (matmul dtype parity with the bf16 rows), so
    # use the bf16-ROUNDED node positions and build least-squares
    # values->monomial maps consistent with those exact nodes.
    tqn = np.asarray(TQ * u, dtype=bf16).astype(np.float64)   # g nodes
    tkn = np.asarray(TK * u, dtype=bf16).astype(np.float64)   # f nodes

    def v2mono(nodes_scaled):
        V = np.vander(nodes_scaled, R, increasing=True)       # [NCH, R]
        return np.linalg.pinv(V)                              # [R, NCH]

    Kq = v2mono(tqn / TQ)   # g: coeffs in u = q/TQ from values at tqn/TQ
    Kk = v2mono(tkn / TK)   # f: coeffs in u = k/TK from values at tkn/TK
    krhs = np.zeros((128, 2 * R), np.float32)  # [(i,m), j] = K[j, m]; g|f
    mask01 = np.zeros((128, 128), np.float32)  # [(i,m),(i',pp)] = (i==i')
    tqmask = np.zeros((BL, 128), bf16)         # [i',(i,m)] = (i==i')*tqn_m
    tkmask = np.zeros((BL, 128), bf16)
    bmask = np.zeros((BL, 128), bf16)          # [i',(i,m)] = (i==i')
    for i in range(BL):
        krhs[32 * i:32 * i + 32, 0:R] = Kq.T.astype(np.float32)
        krhs[32 * i:32 * i + 32, R:2 * R] = Kk.T.astype(np.float32)
        mask01[32 * i:32 * i + 32, 32 * i:32 * i + 32] = 1.0
        tqmask[i, 32 * i:32 * i + 32] = tqn.astype(bf16)
        tkmask[i, 32 * i:32 * i + 32] = tkn.astype(bf16)
        bmask[i, 32 * i:32 * i + 32] = 1.0
    return krhs, mask01, tqmask, tkmask, bmask


def _build():
    nc = bacc.Bacc("TRN2", target_bir_lowering=False, debug=False,
                   num_devices=NCORES)
    # tiled layouts: per-partition contiguous
    xT_d = nc.dram_tensor("xT", [128, KCH * B], DT.bfloat16,
                          kind="ExternalInput")
    w_d = nc.dram_tensor("w", [128, KCH * 3 * SL], DT.float8e4,
                         kind="ExternalInput")
    # packed consts: bf16 row (ones|bias), f32 block (krhs|mask01|xp4),
    # bf16 mask block (tqm|tkm|bm)
    cb_d = nc.dram_tensor("cb", [1, B + 3 * SL], DT.bfloat16,
                          kind="ExternalInput")
    cf_d = nc.dram_tensor("cf", [128, 2 * R + 128 + 64], DT.float32,
                          kind="ExternalInput")
    cm_d = nc.dram_tensor("cm", [BL, 3 * 128], DT.bfloat16,
                          kind="ExternalInput")
    out_d = nc.dram_tensor("out", [BL, SEQ], DT.float32, kind="ExternalOutput")

    cc_in = nc.dram_tensor("cc_in", [B, 3 * SL], DT.bfloat16)
    cc_out = nc.dram_tensor("cc_out", [B, 3 * SL], DT.bfloat16)

    H = SEQ // 2
    Q = SEQ // 4

    with tile.TileContext(nc) as tc, ExitStack() as ctx:
        pool = ctx.enter_context(tc.tile_pool(name="main", bufs=1))

        # ---- phase 1 loads ------------------------------------------------
        # consts first (tiny transfers; DMA_ENGINES is a serial device and
        # the bias matmul needs cb before the W stream monopolizes it)
        cb_t = pool.tile([1, B + 3 * SL], DT.bfloat16)
        nc.sync.dma_start(cb_t[:], cb_d.ap())
        # x: one HWDGE dma (contiguous per partition)
        xt = pool.tile([128, KCH * B], DT.bfloat16)
        nc.sync.dma_start(xt[:], xT_d.ap())
        cf_t = pool.tile([128, 2 * R + 128 + 64], DT.float32)
        nc.sync.dma_start(cf_t[:], cf_d.ap())
        cm_t = pool.tile([BL, 3 * 128], DT.bfloat16)
        nc.sync.dma_start(cm_t[:], cm_d.ap())
        ones_t = cb_t[:, 0:B]
        bias_t = cb_t[:, B:B + 3 * SL]
        krhs_t = cf_t[:, 0:2 * R]
        mask_t = cf_t[:, 2 * R:2 * R + 128]
        xp4 = cf_t[:, 2 * R + 128:2 * R + 192]
        tqm_t = cm_t[:, 0:128]
        tkm_t = cm_t[:, 128:256]
        bm_t = cm_t[:, 256:384]

        # W: 6 chunks (tiny tail chunks so the last matmul's data lands
        # early), alternating Pool-SWDGE / ACT-HWDGE queues; DMA_ENGINES
        # streams them serially at ~360GB/s
        wall = pool.tile([128, KCH * 3 * SL], DT.float8e4)
        for (k0, k1), eng in (((0, 4), nc.gpsimd), ((4, 8), nc.scalar),
                              ((8, 11), nc.gpsimd), ((11, 14), nc.scalar),
                              ((14, 15), nc.gpsimd), ((15, 16), nc.scalar)):
            eng.dma_start(wall[:, k0 * 768:k1 * 768],
                          w_d.ap()[:, k0 * 768:k1 * 768])

        warm = pool.tile([1, 1], DT.float32)
        nc.scalar.activation(warm[:], ones_t[0:1, 0:1], F.Exp)

        # PE pre-warm on a memset scratch so the projection matmuls price at
        # full clock (p-state is locked at visit; cold PE runs 3.7x slower)
        scratch = pool.tile([128, SL], DT.bfloat16)
        nc.vector.memset(scratch[:], 1.0)
        warm_ctx = ExitStack()
        pwx = warm_ctx.enter_context(tc.tile_pool(name="pswarm", bufs=1,
                                                  space="PSUM"))
        scr = pwx.tile([B, SL], DT.float32)
        for d in range(N_WARM0):
            nc.tensor.matmul(scr[:], scratch[:, 0:B], scratch[:],
                             start=(d == 0), stop=False)

        # ---- phase 1 compute: bias first, then merged q|k|v per chunk ----
        cvt = pool.tile([B, 3 * SL], DT.bfloat16)
        with tc.tile_pool(name="psp", bufs=1, space="PSUM") as pp:
            ps_qk = pp.tile([B, 2 * SL], DT.float32)
            ps_v = pp.tile([B, SL], DT.float32)
            nc.tensor.matmul(ps_qk[:], ones_t[:], bias_t[:, 0:2 * SL],
                             start=True, stop=False)
            nc.tensor.matmul(ps_v[:], ones_t[:], bias_t[:, 2 * SL:3 * SL],
                             start=True, stop=False)
            for kc in range(KCH):
                xk = xt[:, kc * B:(kc + 1) * B]
                nc.tensor.matmul(ps_qk[:], xk,
                                 wall[:, kc * 768:kc * 768 + 512],
                                 start=False, stop=(kc == KCH - 1))
                nc.tensor.matmul(ps_v[:], xk,
                                 wall[:, kc * 768 + 512:(kc + 1) * 768],
                                 start=False, stop=(kc == KCH - 1))
            # parallel converts (undo the x64 fp8 weight scaling)
            nc.scalar.activation(cvt[:, 2 * SL:3 * SL], ps_v[:], F.Copy,
                                 scale=1.0 / 64.0)
            nc.vector.tensor_scalar(cvt[:, 0:2 * SL], ps_qk[:], 1.0 / 64.0,
                                    None, op0=OP.mult)
        nc.sync.dma_start(cc_in.ap(), cvt[:])
        nc.gpsimd.collective_compute(
            "AllToAll", OP.bypass, replica_groups=[list(range(NCORES))],
            ins=[cc_in.ap()], outs=[cc_out.ap()])

        # keep PE hot through the collective window so post-collective
        # matmuls are costed at full clock (p-state is locked at visit time)
        for d in range(N_WARM1):
            nc.tensor.matmul(scr[:], scratch[:, 0:B], scratch[:],
                             start=False, stop=(d == N_WARM1 - 1))
        warm_ctx.close()

        # ---- phase 2: one batch-major cc load; broadcasts read it direct --
        # cc2[i, (d, o)] = cc_out[(d, i), o]: every per-source-core slice
        # starts at partition 0 (PE rhs base-partition restriction)
        cc2 = pool.tile([BL, NCORES * 3 * SL], DT.bfloat16)
        nc.sync.dma_start(
            cc2[:].rearrange("i (d o) -> i d o", d=NCORES),
            cc_out.ap().rearrange("(d i) o -> i d o", i=BL))
        # point layouts per-batch direct from DRAM (3D in-APs; a single 4D
        # AP pairing is rejected by the 3-dim DMA balancing limit).
        # p4[(i, pp), f] = row[i, pp*64+f] with pp = d*4 + o4.
        # (ACT issues NO dmas here: its SEQ must be free to dispatch the exps
        # the moment karg is ready)
        cco = cc_out.ap()
        qp4 = pool.tile([128, 64], DT.bfloat16)
        for i in range(BL):
            nc.sync.dma_start(
                qp4[32 * i:32 * i + 32, :],
                cco[:, 0:SL].rearrange("(d i) (o4 f) -> i d o4 f",
                                       i=BL, f=64)[i])
        v4 = pool.tile([128, 64], DT.bfloat16)
        for i in range(BL):
            nc.gpsimd.dma_start(
                v4[32 * i:32 * i + 32, :],
                cco[:, 2 * SL:3 * SL].rearrange("(d i) (o4 f) -> i d o4 f",
                                                i=BL, f=64)[i])
        kp4 = pool.tile([128, 64], DT.bfloat16)
        for i in range(BL):
            nc.gpsimd.dma_start(
                kp4[32 * i:32 * i + 32, :],
                cco[:, SL:2 * SL].rearrange("(d i) (o4 f) -> i d o4 f",
                                            i=BL, f=64)[i])

        # u and s = u^2 tiles
        uq = pool.tile([128, 64], DT.float32)
        nc.vector.tensor_scalar(uq[:], qp4[:], 1.0 / TQ, None, op0=OP.mult)
        sq = pool.tile([128, 64], DT.float32)
        nc.vector.tensor_mul(sq[:], uq[:], uq[:])
        uk = pool.tile([128, 64], DT.float32)
        nc.vector.tensor_scalar(uk[:], kp4[:], 1.0 / TK, None, op0=OP.mult)
        sk = pool.tile([128, 64], DT.float32)
        nc.vector.tensor_mul(sk[:], uk[:], uk[:])

        def horner(co, s, u, extra, name):
            """P(u) = sum_j co_j u^j, even/odd split; adds `extra` if given."""
            te = pool.tile([128, 64], DT.float32, name=f"te_{name}")
            to = pool.tile([128, 64], DT.float32, name=f"to_{name}")
            nc.vector.tensor_scalar(te[:], s[:], co[:, 6:7], None,
                                    op0=OP.mult)
            nc.vector.tensor_scalar(to[:], s[:], co[:, 7:8], None,
                                    op0=OP.mult)
            for j in (4, 2):
                nc.vector.scalar_tensor_tensor(
                    te[:], te[:], co[:, j:j + 1], s[:], OP.add, OP.mult)
                nc.vector.scalar_tensor_tensor(
                    to[:], to[:], co[:, j + 1:j + 2], s[:], OP.add, OP.mult)
            nc.vector.scalar_tensor_tensor(
                to[:], to[:], co[:, 1:2], u[:], OP.add, OP.mult)
            res = pool.tile([128, 64], DT.float32, name=f"res_{name}")
            if extra is None:
                nc.vector.tensor_scalar(te[:], te[:], co[:, 0:1], None,
                                        op0=OP.add)
            else:
                nc.vector.scalar_tensor_tensor(
                    te[:], te[:], co[:, 0:1], extra[:], OP.add, OP.add)
            nc.vector.tensor_add(res[:], te[:], to[:])
            return res

        gscr = pool.tile([128, SEQ], DT.bfloat16)
        gv = pool.tile([128, 1], DT.float32)
        p4 = pool.tile([128, SEQ], DT.bfloat16)
        fscr = pool.tile([128, SEQ], DT.bfloat16)
        fvq = [pool.tile([128, 1], DT.float32, name=f"fvq{q}")
               for q in range(4)]
        fva = pool.tile([128, 1], DT.float32)
        fvb = pool.tile([128, 1], DT.float32)
        fv = pool.tile([128, 1], DT.float32)

        with tc.tile_pool(name="psbig", bufs=1, space="PSUM") as pb:
            karg = pb.tile([128, SEQ], DT.float32)
            qarg = pb.tile([128, SEQ], DT.float32)
            # arg[(i,m), j] = t_m * row_i[j]: per-source-core outer products
            # reading cc_sb directly (k slice = cols SL..2SL, q = 0..SL)
            for d in range(NCORES):
                nc.tensor.matmul(karg[:, d * SL:(d + 1) * SL], tqm_t,
                                 cc2[:, d * 768 + SL:d * 768 + 2 * SL],
                                 start=True, stop=True)
            for d in range(NCORES):
                nc.tensor.matmul(qarg[:, d * SL:(d + 1) * SL], tkm_t,
                                 cc2[:, d * 768:d * 768 + SL],
                                 start=True, stop=True)
            # g node values: gv[(i,m)] = sum_j exp(karg)
            nc.scalar.activation(gscr[:], karg[:], F.Exp, accum_out=gv[:])
            # f exp table
            nc.scalar.activation(p4[:], qarg[:], F.Exp)

            # ---- g: Z at q-points, w = v/Z --------------------------------
            gvm = pool.tile([128, 128], DT.float32)
            nc.vector.tensor_scalar(gvm[:], mask_t, gv[:, 0:1], None,
                                    op0=OP.mult)
            cog = pool.tile([128, R], DT.float32)
            # mono matmul lands in spare karg columns (gexp already read them)
            nc.tensor.matmul(karg[:, SEQ - R:SEQ], gvm[:], krhs_t[:, 0:R],
                             start=True, stop=True)
            nc.vector.tensor_copy(cog[:], karg[:, SEQ - R:SEQ])

            # keep PE hot until the w broadcast (deterministic LOW pricing)
            for d in range(N_WARM2):
                nc.tensor.matmul(karg[:, 0:SL], tqm_t, cc2[:, SL:2 * SL],
                                 start=True, stop=True)

            zt = horner(cog, sq, uq, None, "g")
            rz = pool.tile([128, 64], DT.float32)
            nc.vector.reciprocal(rz[:], zt[:])
            wt = pool.tile([128, 64], DT.bfloat16)
            nc.vector.tensor_mul(wt[:], v4[:], rz[:])

            # w: point -> row layout (one SBUF->SBUF hop) -> PE broadcast
            # into the karg banks (gexp is done with them)
            wflat = pool.tile([BL, SEQ], DT.bfloat16)
            nc.sync.dma_start(wflat[:], wt[:])
            for q in range(4):
                nc.tensor.matmul(karg[:, q * Q:(q + 1) * Q], bm_t,
                                 wflat[:, q * Q:(q + 1) * Q],
                                 start=True, stop=True)
            # fv[(i,m)] = sum_j p4 * w4: four DVE chunks pipelined behind the
            # w-broadcast quarters (Pool cannot read PSUM), then combine
            for q in range(4):
                nc.vector.scalar_tensor_tensor(
                    fscr[:, q * Q:(q + 1) * Q], p4[:, q * Q:(q + 1) * Q],
                    1.0, karg[:, q * Q:(q + 1) * Q],
                    OP.mult, OP.mult, accum_out=fvq[q][:])
            nc.vector.tensor_add(fva[:], fvq[0][:], fvq[1][:])
            nc.vector.tensor_add(fvb[:], fvq[2][:], fvq[3][:])
            nc.vector.tensor_add(fv[:], fva[:], fvb[:])

            # f mono coeffs via the same spare-column trick (qarg this time)
            fvm = pool.tile([128, 128], DT.float32)
            nc.vector.tensor_scalar(fvm[:], mask_t, fv[:, 0:1], None,
                                    op0=OP.mult)
            cof = pool.tile([128, R], DT.float32)
            nc.tensor.matmul(qarg[:, SEQ - R:SEQ], fvm[:],
                             krhs_t[:, R:2 * R], start=True, stop=True)
            nc.vector.tensor_copy(cof[:], qarg[:, SEQ - R:SEQ])

        # ---- f: sa at k-points + residual ---------------------------------
        so = horner(cof, sk, uk, xp4, "f")
        nc.sync.dma_start(
            out_d.ap().rearrange("i (pp f) -> (i pp) f", f=64), so[:])
    nc.compile()
    return nc


def _prep_inputs(x, Wq, bq, Wk, bk, Wv, bv):
    import ml_dtypes
    bf16 = ml_dtypes.bfloat16
    fp8 = ml_dtypes.float8_e4m3
    x = np.ascontiguousarray(x, dtype=np.float32)
    # tiled xT: [p, kc*B+m] = x[m, kc*128+p]
    xT = np.ascontiguousarray(
        x.T.astype(bf16).reshape(KCH, 128, B).transpose(1, 0, 2)
        .reshape(128, KCH * B))
    krhs, mask01, tqmask, tkmask, bmask = _consts()
    ones = np.ones((1, B), dtype=bf16)
    cm = np.ascontiguousarray(
        np.concatenate([tqmask, tkmask, bmask], axis=1))
    in_maps = []
    for c in range(NCORES):
        sl = slice(SL * c, SL * (c + 1))
        w_all = np.concatenate([Wq[sl].T, Wk[sl].T, Wv[sl].T], axis=1)
        w_tiled = np.ascontiguousarray(
            (w_all * 64.0).astype(fp8).reshape(KCH, 128, 3 * SL)
            .transpose(1, 0, 2).reshape(128, KCH * 3 * SL))
        bias = np.concatenate([bq[sl], bk[sl], bv[sl]])[None, :]
        cb = np.ascontiguousarray(np.concatenate(
            [ones, (bias * 64.0).astype(bf16)], axis=1, dtype=bf16))
        xloc = x[BL * c:BL * (c + 1)]
        xp4 = np.ascontiguousarray(xloc.reshape(128, 64))
        cf = np.ascontiguousarray(
            np.concatenate([krhs, mask01, xp4], axis=1, dtype=np.float32))
        in_maps.append({
            "xT": xT,
            "w": w_tiled,
            "cb": cb,
            "cf": cf,
            "cm": cm,
        })
    return in_maps


def run_on_device(x, Wq, bq, Wk, bk, Wv, bv, **spmd_kwargs):
    if "nc" not in _CACHE:
        _CACHE["nc"] = _build()
    nc = _CACHE["nc"]
    in_maps = _prep_inputs(x, Wq, bq, Wk, bk, Wv, bv)
    res = run_bass_kernel_spmd(nc, in_maps, core_ids=list(range(NCORES)),
                               **spmd_kwargs)
    out = np.concatenate([res.results[c]["out"] for c in range(NCORES)], axis=0)
    return np.ascontiguousarray(out, dtype=np.float32), res


def kernel(x, Wq, bq, Wk, bk, Wv, bv):
    out, _ = run_on_device(x, Wq, bq, Wk, bk, Wv, bv)
    return out
